# revision 1
# baseline (speedup 1.0000x reference)
"""Trainium2 Bass kernel for nn_EnhancedQuantumLayer (6-qubit circuit, B=32768).

Reduction: the circuit is AngleEmbedding (per-sample RX product state) followed
by a batch-independent 64x64 unitary U (StronglyEntanglingLayers + CNOT rings,
function of `weights` only), then per-qubit PauliZ expectations.

Per sample b:
    m_b   = kron_q [cos(a_q/2), sin(a_q/2)]           (real 64-vec, a = x*scale)
    A_b   = [Re(Cc^T) ; Im(Cc^T)] m_b                 (128-vec; Cc folds the
                                                       (-i)^popcount embedding
                                                       phases into U)
    EV_bq = sum_p sgn2[p,q] * A_b[p]^2                (signs of PauliZ)

Device work per core (4096 samples): one fused input DMA -> one fat ACT Sin
(cos|sin|w halves) -> DVE kron via 5 fused broadcast-AP multiplies building M
in a 32x32 block-swizzled layout -> one DVE StreamTranspose (= M_T, basis on
partitions; zero PE transposes) -> 8 packed 64->128 matmuls -> 2 fat ACT
squares -> 8 sign matmuls -> fat EV copy -> 2 output DMAs. Host does the tiny
weights->matrix precompute, the lane permutation/bias pre-add of the input,
and the inverse column permutation of the output.
"""
import math
from contextlib import ExitStack

import numpy as np

import concourse.bass as bass
import concourse.mybir as mybir
from concourse.bass_utils import run_bass_kernel_spmd

F32 = mybir.dt.float32
NQ = 6
NL = 6
B = 32768
NCORES = 8
BC = B // NCORES          # 4096 samples per core
NSB = 64                  # angle blocks per lane (s, t', p_hi)


# ---------------------------------------------------------------- host precompute
def _host_matrices(weights):
    """(CcPacked (64,128) f32, SgnZ2 (128,6) f32) from weights (6,6,3)."""
    w = np.asarray(weights, dtype=np.float64)
    phi, theta, omega = w[..., 0], w[..., 1], w[..., 2]
    ct, st = np.cos(0.5 * theta), np.sin(0.5 * theta)
    em = np.exp(-0.5j * (phi + omega))
    ep = np.exp(0.5j * (phi + omega))
    epm = np.exp(0.5j * (phi - omega))
    emp = np.exp(-0.5j * (phi - omega))

    state = np.eye(64, dtype=np.complex128).reshape((64,) + (2,) * NQ)

    def apply_1q(state, U, q):
        ax = q + 1
        s = np.moveaxis(state, ax, -1)
        s = np.einsum('ij,...j->...i', U, s)
        return np.moveaxis(s, -1, ax)

    def cnot(state, c, t):
        ca, ta = c + 1, t + 1
        s0 = np.take(state, 0, axis=ca)
        s1 = np.take(state, 1, axis=ca)
        t_in = ta - 1 if ta > ca else ta
        s1 = np.flip(s1, axis=t_in)
        return np.stack([s0, s1], axis=ca)

    for l in range(NL):
        for q in range(NQ):
            U = np.array([
                [em[l, q] * ct[l, q], -epm[l, q] * st[l, q]],
                [emp[l, q] * st[l, q], ep[l, q] * ct[l, q]],
            ])
            state = apply_1q(state, U, q)
        r = (l % (NQ - 1)) + 1
        for q in range(NQ):
            state = cnot(state, q, (q + r) % NQ)

    stateF = state.reshape(64, 64)            # [in_e, out_o] = U[o, e]
    e = np.arange(64)
    pc = np.array([bin(v).count('1') for v in e])
    phase = (-1j) ** pc                       # (-i)^popcount: RX embedding phases
    Cc = phase[:, None] * stateF              # (64_in, 64_out)

    # device row j has qubit q at bit q; reference index e has qubit 0 as MSB
    bitrev = np.array([int(format(j, '06b')[::-1], 2) for j in range(64)])
    Cdev = Cc[bitrev, :]

    ccpacked = np.concatenate([Cdev.real, Cdev.imag], axis=1)   # (64, 128)

    o = np.arange(64)
    z = np.stack([1.0 - 2.0 * ((o >> (5 - q)) & 1) for q in range(NQ)], axis=1)
    sgn2 = np.concatenate([z, z], axis=0)                        # (128, 6)
    return ccpacked.astype(np.float32), sgn2.astype(np.float32)


def _out_perm():
    """col g in device out (6, BC) holds sample_local perm[g]."""
    perm = np.empty(BC, np.int64)
    for j in range(8):
        h, s = j // 4, j % 4
        c = np.arange(512)
        tp = c // 128
        p_hi = (c % 128) // 32
        pl = c % 32
        perm[j * 512 + c] = 1024 * p_hi + 32 * pl + 8 * s + 2 * tp + h
    return perm


def _lane_sample_index():
    """SL[L, sb]: sample_local for lane L, angle-block sb."""
    L = np.arange(128)
    h, jh, pl = L >> 6, (L >> 5) & 1, L & 31
    sb = np.arange(64)
    s, tp, p_hi = sb >> 4, (sb >> 2) & 3, sb & 3
    return (1024 * p_hi[None, :] + 32 * pl[:, None]
            + 8 * s[None, :] + 2 * tp[None, :] + h[:, None])


_SL = _lane_sample_index()


# ---------------------------------------------------------------- device program
def _build_bass(reps=1):
    """Raw-bass pipeline, ~30 instructions per core per iteration.

    Layout trick: the kron product M is built directly in a 32x32
    block-swizzled layout (lane = (group-parity h, qubit-5 bit, sample
    low bits); host permutes the input accordingly), so a single DVE
    StreamTranspose yields M_T with basis index on partitions - no PE
    transposes at all. Per iteration:

      SP    in-DMA, 2 out-DMAs
      ACT   ONE fused Sin over (128,832) (host pre-adds pi/2 and the
            per-lane qubit-5 bias into duplicated angle columns),
            2 fat squares, 1 fat EV copy
      DVE   5 fused kron ops (3D broadcast APs), 1 StreamTranspose
      PE    8 packed 64->128 matmuls, 8 sign matmuls (packed into
            PSUM partition bases {0,64})
    """
    nc = bass.Bass()
    # xin cols: [angles+pi/2 0:384 | angles 384:768 | a5+wbias 768:832
    #            | Cc packed twice 832:960 | sgn2 960:966]
    xin = nc.dram_tensor("xin", [128, 966], F32, kind="ExternalInput")
    out = nc.dram_tensor("out", [NQ, BC], F32, kind="ExternalOutput")

    AT, VT, PT = 4, 6, 16

    ctx = ExitStack()
    with ctx:
        sb = lambda nm, shape: ctx.enter_context(nc.sbuf_tensor(nm, shape, F32))
        ps = lambda nm, shape: ctx.enter_context(nc.psum_tensor(nm, shape, F32))
        sem = lambda nm: ctx.enter_context(nc.semaphore(name=nm))

        xt = sb("xt", [128, 966])
        scs = sb("scs", [128, 832])       # cos | sin | w
        k1b = sb("k1b", [128, 256])
        k2b = sb("k2b", [128, 256])
        k3b = sb("k3b", [128, 128])
        m12b = sb("m12b", [128, 1024])
        mswz = sb("mswz", [128, 2048])
        mtall = sb("mtall", [128, 2048])
        ppb = sb("ppb", [128, 4096])
        evst = sb("evst", [128, 2048])
        amall = ps("amall", [128, 2048])
        am2 = ps("am2", [128, 2048])

        Sd, Sa, Sv, Sp, So = (sem("Sd"), sem("Sa"), sem("Sv"), sem("Sp"),
                              sem("So"))

        cc2 = xt.ap()[:, 832:960]
        sg_t = xt.ap()[:, 960:966]

        # 3-free-dim views for the fused kron
        def hsq(q):
            # (p, sb, hf, 1) -> pick angle q, hf = cos/sin half (step 384)
            return (scs.ap()[:, 0:768]
                    .rearrange("p (hf sb q) -> p sb hf q", hf=2, q=NQ)
                    [:, :, :, q:q + 1])

        block = ctx.enter_context(nc.Block())

        @block.sync
        def _(sync):
            for i in range(reps):
                d = sync.dma_start(out=xt.ap()[:, :], in_=xin[:, :])
                if i > 0:
                    d._wait_ge(Sp, PT * i)
                d.then_inc(Sd, 16)
                for b in range(2):
                    dst = (out.rearrange("q (jj bb c) -> q jj bb c",
                                         bb=2, c=512)[:, :, b, :])
                    o = sync.dma_start(
                        out=dst,
                        in_=evst.ap()[64 * b:64 * b + NQ, :]
                            .rearrange("q (jj c) -> q jj c", c=512),
                    )
                    o._wait_ge(Sa, AT * (i + 1)).then_inc(So, 16)
            sync.wait_ge(So, 32 * reps)

        @block.scalar
        def _(scalar):
            sfn = mybir.ActivationFunctionType.Sin
            sqf = mybir.ActivationFunctionType.Square
            for i in range(reps):
                ab, pb = AT * i, PT * i
                c_ = nc.scalar.activation(
                    scs.ap()[:, 0:832], xt.ap()[:, 0:832], sfn,
                )
                c_._wait_ge(Sd, 16 * (i + 1)).then_inc(Sa, 1)
                q1 = nc.scalar.activation(ppb.ap()[:, 0:2048],
                                          amall.ap()[:, :], sqf)
                q1._wait_ge(Sp, pb + 4).then_inc(Sa, 1)
                q2 = nc.scalar.activation(ppb.ap()[:, 2048:4096],
                                          am2.ap()[:, :], sqf)
                q2._wait_ge(Sp, pb + 8).then_inc(Sa, 1)
                e_ = nc.scalar.copy(evst.ap()[:, :], amall.ap()[:, :])
                e_._wait_ge(Sp, pb + 16).then_inc(Sa, 1)

        @block.vector
        def _(vector):
            for i in range(reps):
                ab, vb, pb = AT * i, VT * i, PT * i
                # k1 = t0 (x) t1
                o1 = (k1b.ap()[:, :]
                      .rearrange("p (sb b1 b0) -> p sb b1 b0", b1=2, b0=2))
                i0 = hsq(0).squeeze(3).unsqueeze(2).broadcast_to((128, 64, 2, 2))
                i1 = hsq(1).squeeze(3).unsqueeze(3).broadcast_to((128, 64, 2, 2))
                t = nc.vector.tensor_mul(o1, i0, i1)
                t._wait_ge(Sa, ab + 1).then_inc(Sv, 1)
                o2 = (k2b.ap()[:, :]
                      .rearrange("p (sb b3 b2) -> p sb b3 b2", b3=2, b2=2))
                i0 = hsq(2).squeeze(3).unsqueeze(2).broadcast_to((128, 64, 2, 2))
                i1 = hsq(3).squeeze(3).unsqueeze(3).broadcast_to((128, 64, 2, 2))
                t = nc.vector.tensor_mul(o2, i0, i1)
                if i > 0:
                    t._wait_ge(Sp, pb - PT + 8)
                t.then_inc(Sv, 1)
                o3 = (k3b.ap()[:, :]
                      .rearrange("p (sb b4) -> p sb b4", b4=2))
                i0 = hsq(4).squeeze(3)
                i1 = (scs.ap()[:, 768:832]
                      .rearrange("p (sb o) -> p sb o", o=1)
                      .broadcast_to((128, 64, 2)))
                t = nc.vector.tensor_mul(o3, i0, i1)
                if i > 0:
                    t._wait_ge(So, 32 * i)    # evst free (out-DMAs i-1)
                t.then_inc(Sv, 1)
                om = (m12b.ap()[:, :]
                      .rearrange("p (sb b32 b10) -> p sb b32 b10", b32=4, b10=4))
                i0 = (k1b.ap()[:, :].rearrange("p (sb w) -> p sb w", w=4)
                      .unsqueeze(2).broadcast_to((128, 64, 4, 4)))
                i1 = (k2b.ap()[:, :].rearrange("p (sb w) -> p sb w", w=4)
                      .unsqueeze(3).broadcast_to((128, 64, 4, 4)))
                t = nc.vector.tensor_mul(om, i0, i1)
                t._wait_ge(Sv, vb + 2).then_inc(Sv, 1)
                oM = (mswz.ap()[:, :]
                      .rearrange("p (sb b4 w) -> p sb b4 w", b4=2, w=16))
                i0 = (m12b.ap()[:, :].rearrange("p (sb w) -> p sb w", w=16)
                      .unsqueeze(2).broadcast_to((128, 64, 2, 16)))
                i1 = (k3b.ap()[:, :].rearrange("p (sb b4) -> p sb b4", b4=2)
                      .unsqueeze(3).broadcast_to((128, 64, 2, 16)))
                t = nc.vector.tensor_mul(oM, i0, i1)
                t._wait_ge(Sv, vb + 4).then_inc(Sv, 1)
                st = nc.vector.transpose(mtall.ap()[:, :], mswz.ap()[:, :])
                st._wait_ge(Sv, vb + 5).then_inc(Sv, 1)

        @block.tensor
        def _(tensor):
            for i in range(reps):
                ab, vb = AT * i, VT * i
                for k in range(8):
                    h, s = divmod(k, 4)
                    dst = [amall, am2][h]
                    mm = nc.tensor.matmul(
                        dst.ap()[:, s * 512:(s + 1) * 512],
                        cc2[64 * h:64 * h + 64, :],
                        mtall.ap()[64 * h:64 * h + 64, s * 512:(s + 1) * 512],
                        start=True, stop=True,
                    )
                    if k == 0:
                        mm._wait_ge(Sv, vb + 6)
                    if k in (3, 7):
                        mm.then_inc(Sp, 4)
                for j in range(8):
                    mm = nc.tensor.matmul(
                        amall.ap()[64 * (j % 2):64 * (j % 2) + NQ,
                                   (j // 2) * 512:(j // 2) * 512 + 512],
                        sg_t, ppb.ap()[:, j * 512:(j + 1) * 512],
                        start=True, stop=True,
                    )
                    if j == 0:
                        mm._wait_ge(Sa, ab + 2)
                    if j == 4:
                        mm._wait_ge(Sa, ab + 3)
                    if j == 7:
                        mm.then_inc(Sp, 8)

    return nc


_CACHE = {}


def _get_nc():
    if "nc" not in _CACHE:
        _CACHE["nc"] = _build_bass()
        _CACHE["perm"] = _out_perm()
    return _CACHE["nc"], _CACHE["perm"]


# ---------------------------------------------------------------- entry point
def _make_in_maps(x, weights, scale):
    x = np.asarray(x, dtype=np.float32)
    ccp, sg2 = _host_matrices(weights)
    hs = 0.5 * float(np.asarray(scale).reshape(-1)[0])
    a = x * hs                                   # (B, 6) half-angles
    L = np.arange(128)
    wbias = np.where(((L >> 5) & 1) == 0, math.pi / 2, 0.0).astype(np.float32)
    in_maps = []
    for k in range(NCORES):
        ak = a[k * BC:(k + 1) * BC]              # (4096, 6)
        lane = ak[_SL].reshape(128, 384)
        xs2 = np.empty((128, 966), np.float32)
        xs2[:, 0:384] = lane + np.float32(math.pi / 2)
        xs2[:, 384:768] = lane
        xs2[:, 768:832] = lane[:, 5::6] + wbias[:, None]
        xs2[0:64, 832:960] = ccp
        xs2[64:128, 832:960] = ccp
        xs2[:, 960:966] = sg2
        in_maps.append({"xin": xs2})
    return in_maps


def kernel(x, weights, scale):
    nc, perm = _get_nc()
    in_maps = _make_in_maps(x, weights, scale)
    res = run_bass_kernel_spmd(nc, in_maps, list(range(NCORES))).results
    ev = np.empty((B, NQ), np.float32)
    for k in range(NCORES):
        ev[k * BC + perm, :] = res[k]["out"].T
    return ev


if __name__ == "__main__":
    rng = np.random.default_rng(0)
    x = rng.standard_normal((B, NQ)).astype(np.float32)
    weights = rng.uniform(0, 2 * np.pi, (NL, NQ, 3)).astype(np.float32)
    scale = np.array([0.1], np.float32)
    ev = kernel(x, weights, scale)
    print("out", ev.shape, ev.dtype, ev[:2])



# revision 4
# speedup vs baseline: 48.1633x; 48.1633x over previous
"""Trainium2 Bass kernel for nn_EnhancedQuantumLayer (6-qubit circuit, B=32768).

Reduction: AngleEmbedding (per-sample RX product state) + batch-independent
64x64 unitary U (weights only) + per-qubit PauliZ expectations:

    m_b   = kron_q [cos(a_q/2), sin(a_q/2)]           (real 64-vec, a = x*scale)
    A_b   = Cstat^T m_b                                (128-vec, re/im packed)
    EV_bq = sum_p sgn[p,q] * A_b[p]^2

Instruction-count-minimized pipeline (the platform charges a ~fixed cost per
engine instruction): per rep and core (4096 samples):
  sync : in-DMA (128,704), out-DMA (6,4096)
  ACT  : 1 fat Sin, 1 fat Square (PSUM->SBUF), 1 fat EV copy
  DVE  : k12 (fused pair-kron), k3, m12, mswz, StreamTranspose
  PE   : 8 projection matmuls + 8 sign matmuls (PSUM bank limit: 512 cols)
Constants (projection matrix, signs) are DMA'd once outside the rep loop.
"""
import math
from contextlib import ExitStack

import numpy as np

import concourse.bass as bass
import concourse.mybir as mybir
from concourse.bass_utils import run_bass_kernel_spmd

F32 = mybir.dt.float32
NQ = 6
NL = 6
B = 32768
NCORES = 8
BC = B // NCORES


# ---------------------------------------------------------------- host precompute
def _unitary64(weights):
    """Cc (64,64) complex: folded RX-embedding phases + circuit unitary."""
    w = np.asarray(weights, dtype=np.float64)
    phi, theta, omega = w[..., 0], w[..., 1], w[..., 2]
    ct, st = np.cos(0.5 * theta), np.sin(0.5 * theta)
    em = np.exp(-0.5j * (phi + omega))
    ep = np.exp(0.5j * (phi + omega))
    epm = np.exp(0.5j * (phi - omega))
    emp = np.exp(-0.5j * (phi - omega))

    state = np.eye(64, dtype=np.complex128).reshape((64,) + (2,) * NQ)

    def apply_1q(state, U, q):
        ax = q + 1
        s = np.moveaxis(state, ax, -1)
        s = np.einsum('ij,...j->...i', U, s)
        return np.moveaxis(s, -1, ax)

    def cnot(state, c, t):
        ca, ta = c + 1, t + 1
        s0 = np.take(state, 0, axis=ca)
        s1 = np.take(state, 1, axis=ca)
        t_in = ta - 1 if ta > ca else ta
        s1 = np.flip(s1, axis=t_in)
        return np.stack([s0, s1], axis=ca)

    for l in range(NL):
        for q in range(NQ):
            U = np.array([
                [em[l, q] * ct[l, q], -epm[l, q] * st[l, q]],
                [emp[l, q] * st[l, q], ep[l, q] * ct[l, q]],
            ])
            state = apply_1q(state, U, q)
        r = (l % (NQ - 1)) + 1
        for q in range(NQ):
            state = cnot(state, q, (q + r) % NQ)

    stateF = state.reshape(64, 64)
    e = np.arange(64)
    pc = np.array([bin(v).count('1') for v in e])
    phase = (-1j) ** pc
    return phase[:, None] * stateF            # (64_in_ref, 64_out)


def _host_const(weights):
    """cst (128, 134): [Cstat duplicated on both 64-halves | sgn]."""
    Cc = _unitary64(weights)
    # device contraction row j: bit->qubit map {5:q5,4:q4,3:q2,2:q3,1:q0,0:q1}
    j = np.arange(64)
    eref = (((j >> 5) & 1) * 1 + ((j >> 4) & 1) * 2 + ((j >> 3) & 1) * 8
            + ((j >> 2) & 1) * 4 + ((j >> 1) & 1) * 32 + (j & 1) * 16)
    Cdev = Cc[eref, :]                        # (64 j, 64 o)
    Cstat = np.empty((64, 128), np.float64)
    Cstat[:, 0::2] = Cdev.real                # projection p = 2o + 0
    Cstat[:, 1::2] = Cdev.imag                # projection p = 2o + 1
    p = np.arange(128)
    o = p >> 1
    sgn = np.stack([1.0 - 2.0 * ((o >> (5 - q)) & 1) for q in range(NQ)],
                   axis=1)                    # (128, 6)
    cst = np.zeros((128, 134), np.float32)
    cst[0:64, 0:128] = Cstat
    cst[64:128, 0:128] = Cstat
    cst[:, 128:134] = sgn
    return cst


def _lane_sample_index():
    """SL[L, sb]: local sample index held by lane L at angle-block sb."""
    L = np.arange(128)
    h, jh, pl = L >> 6, (L >> 5) & 1, L & 31
    sb = np.arange(64)
    s2, tp, p_hi = sb >> 4, (sb >> 2) & 3, sb & 3
    return (1024 * p_hi[None, :] + 32 * pl[:, None]
            + 8 * s2[None, :] + 2 * tp[None, :] + h[:, None])


def _out_perm():
    """perm[c] = local sample index stored at device out column c."""
    c = np.arange(BC)
    h = c >> 11
    s2 = (c >> 9) & 3
    tp = (c >> 7) & 3
    p_hi = (c >> 5) & 3
    pl = c & 31
    return 1024 * p_hi + 32 * pl + 8 * s2 + 2 * tp + h


_SL = _lane_sample_index()
_PERM = _out_perm()


# ---------------------------------------------------------------- device program
def _build_bass(reps=1):
    """Per-engine hardware loops (Fori): the per-rep pipeline is 26 static
    instructions; iterations synchronize with iteration-indexed semaphore
    thresholds (standalone wait_ge supports register values)."""
    nc = bass.Bass()
    xin = nc.dram_tensor("xin", [128, 704], F32, kind="ExternalInput")
    cin = nc.dram_tensor("cin", [128, 134], F32, kind="ExternalInput")
    out = nc.dram_tensor("out", [NQ, BC], F32, kind="ExternalOutput")

    ctx = ExitStack()
    with ctx:
        sb = lambda nm, shape: ctx.enter_context(nc.sbuf_tensor(nm, shape, F32))
        sem = lambda nm: ctx.enter_context(nc.semaphore(name=nm))

        xt = sb("xt", [128, 704])
        scs = sb("scs", [128, 704])
        k12b = sb("k12b", [128, 512])
        k3b = sb("k3b", [128, 128])
        m12b = sb("m12b", [128, 1024])
        mswz = sb("mswz", [128, 2048])
        mtall = sb("mtall", [128, 2048])
        ppb = sb("ppb", [128, 4096])
        cstb = sb("cstb", [128, 134])
        evo = sb("evo", [NQ, 4096])
        PS = ctx.enter_context(nc.psum_tensor("PS", [128, 4096], F32))

        Sd, Sa, Sv, Sp, So = (sem("Sd"), sem("Sa"), sem("Sv"), sem("Sp"),
                              sem("So"))

        block = ctx.enter_context(nc.Block())

        @block.sync
        def _(sync):
            c0 = sync.dma_start(out=cstb.ap()[:, :], in_=cin[:, :])
            c0.then_inc(Sd, 16)
            with sync.Fori(1, reps + 1) as i:
                sync.wait_ge(Sv, 5 * i - 5)   # k3 of prev iter read xt/scs
                d = sync.dma_start(out=xt.ap()[:, :], in_=xin[:, :])
                d.then_inc(Sd, 16)
                sync.wait_ge(Sa, 3 * i)       # evcopy of this iter done
                o = sync.dma_start(out=out[:, :], in_=evo.ap()[:, :])
                o.then_inc(So, 16)
            sync.wait_ge(So, 16 * reps)

        @block.scalar
        def _(scalar):
            sfn = mybir.ActivationFunctionType.Sin
            sqf = mybir.ActivationFunctionType.Square
            with scalar.Fori(1, reps + 1) as i:
                scalar.wait_ge(Sd, 16 * i + 16)     # cin + in-DMA i done
                s_ = nc.scalar.activation(scs.ap()[:, :], xt.ap()[:, :], sfn)
                s_.then_inc(Sa, 1)
                scalar.wait_ge(Sp, 16 * i - 8)      # A-matmuls of iter i done
                q_ = nc.scalar.activation(ppb.ap()[:, :], PS.ap()[:, :], sqf)
                q_.then_inc(Sa, 1)
                scalar.wait_ge(Sp, 16 * i)          # EV matmuls of iter i done
                scalar.wait_ge(So, 16 * i - 16)     # out-DMA of prev iter done
                e_ = nc.scalar.copy(evo.ap()[:, :], PS.ap()[0:NQ, :])
                e_.then_inc(Sa, 1)

        @block.vector
        def _(vector):
            with vector.Fori(1, reps + 1) as i:
                # k12: fused pair-kron for qubit pairs (0,1) and (2,3)
                vector.wait_ge(Sa, 3 * i - 2)       # sin of iter i done
                v = scs.ap()[:, 0:512].rearrange(
                    "p (hf sbj r) -> p sbj hf r", hf=2, r=2)
                i0 = v[:, :, :, 0:1].broadcast_to((128, 128, 2, 2))
                i1 = (v[:, :, :, 1:2]
                      .rearrange("p sbj hf one -> p sbj one hf")
                      .broadcast_to((128, 128, 2, 2)))
                ok = k12b.ap().rearrange(
                    "p (sbj hf0 hf1) -> p sbj hf0 hf1", hf0=2, hf1=2)
                t = nc.vector.tensor_mul(ok, i0, i1)
                t.then_inc(Sv, 1)
                # k3 = t4 (x) w   (qubit-5 factor via lane-parity bias)
                i0 = scs.ap()[:, 512:640].rearrange("p (hf sb) -> p sb hf",
                                                    hf=2)
                i1 = (scs.ap()[:, 640:704]
                      .rearrange("p (sb one) -> p sb one", one=1)
                      .broadcast_to((128, 64, 2)))
                o3 = k3b.ap().rearrange("p (sb b4) -> p sb b4", b4=2)
                t = nc.vector.tensor_mul(o3, i0, i1)
                t.then_inc(Sv, 1)
                # m12 = k1 (x) k2
                kv = k12b.ap().rearrange("p (sb j w) -> p sb j w", j=2, w=4)
                i0 = kv[:, :, 0:1, :].broadcast_to((128, 64, 4, 4))
                i1 = (kv[:, :, 1:2, :]
                      .rearrange("p sb one w -> p sb w one")
                      .broadcast_to((128, 64, 4, 4)))
                om = m12b.ap().rearrange("p (sb b32 b10) -> p sb b32 b10",
                                         b32=4, b10=4)
                t = nc.vector.tensor_mul(om, i0, i1)
                t.then_inc(Sv, 1)
                # mswz = m12 (x) k3  (block-swizzled for StreamTranspose)
                i0 = (m12b.ap().rearrange("p (sb w) -> p sb w", w=16)
                      .unsqueeze(2).broadcast_to((128, 64, 2, 16)))
                i1 = (k3b.ap().rearrange("p (sb b4) -> p sb b4", b4=2)
                      .unsqueeze(3).broadcast_to((128, 64, 2, 16)))
                oM = mswz.ap().rearrange("p (sb b4 w) -> p sb b4 w",
                                         b4=2, w=16)
                t = nc.vector.tensor_mul(oM, i0, i1)
                t.then_inc(Sv, 1)
                # transpose: basis onto partitions
                vector.wait_ge(Sp, 16 * i - 16)     # prev iter PE read mtall
                st_ = nc.vector.transpose(mtall.ap()[:, :], mswz.ap()[:, :])
                st_.then_inc(Sv, 1)

        @block.tensor
        def _(tensor):
            with tensor.Fori(1, reps + 1) as i:
                tensor.wait_ge(Sv, 5 * i)           # transpose of iter i done
                for k in range(8):
                    h, s4 = divmod(k, 4)
                    mm = nc.tensor.matmul(
                        PS.ap()[:, k * 512:(k + 1) * 512],
                        cstb.ap()[64 * h:64 * h + 64, 0:128],
                        mtall.ap()[64 * h:64 * h + 64,
                                   s4 * 512:(s4 + 1) * 512],
                        start=True, stop=True, skip_group_check=True,
                    )
                    if k == 7:
                        mm.then_inc(Sp, 8)
                tensor.wait_ge(Sa, 3 * i - 1)       # square of iter i done
                for j in range(8):
                    mm = nc.tensor.matmul(
                        PS.ap()[0:NQ, j * 512:(j + 1) * 512],
                        cstb.ap()[:, 128:134],
                        ppb.ap()[:, j * 512:(j + 1) * 512],
                        start=True, stop=True, skip_group_check=True,
                    )
                    if j == 7:
                        mm.then_inc(Sp, 8)

    return nc


_CACHE = {}


def _get_nc():
    if "nc" not in _CACHE:
        _CACHE["nc"] = _build_bass(reps=1)
    return _CACHE["nc"], _PERM


# ---------------------------------------------------------------- entry point
def _make_in_maps(x, weights, scale):
    x = np.asarray(x, dtype=np.float32)
    cst = _host_const(weights)
    hs = 0.5 * float(np.asarray(scale).reshape(-1)[0])
    a = x * hs                                 # half-angles
    L = np.arange(128)
    jh = (L >> 5) & 1
    wbias = np.where(jh == 0, math.pi / 2, 0.0).astype(np.float32)
    HP = np.float32(math.pi / 2)
    in_maps = []
    for k in range(NCORES):
        ak = a[k * BC:(k + 1) * BC]
        ang = ak[_SL]                          # (128, 64, 6)
        xs = np.empty((128, 704), np.float32)
        a4 = ang[:, :, 0:4].reshape(128, 256)  # col = sb*4 + qq
        xs[:, 0:256] = a4 + HP                 # cos half
        xs[:, 256:512] = a4                    # sin half
        xs[:, 512:576] = ang[:, :, 4] + HP
        xs[:, 576:640] = ang[:, :, 4]
        xs[:, 640:704] = ang[:, :, 5] + wbias[:, None]
        in_maps.append({"xin": xs, "cin": cst})
    return in_maps


def kernel(x, weights, scale):
    nc, perm = _get_nc()
    in_maps = _make_in_maps(x, weights, scale)
    res = run_bass_kernel_spmd(nc, in_maps, list(range(NCORES))).results
    ev = np.empty((B, NQ), np.float32)
    for k in range(NCORES):
        ev[k * BC + perm, :] = res[k]["out"].T
    return ev


if __name__ == "__main__":
    rng = np.random.default_rng(0)
    x = rng.standard_normal((B, NQ)).astype(np.float32)
    weights = rng.uniform(0, 2 * np.pi, (NL, NQ, 3)).astype(np.float32)
    scale = np.array([0.1], np.float32)
    ev = kernel(x, weights, scale)
    print("out", ev.shape, ev.dtype, ev[:2])
